# revision 11
# baseline (speedup 1.0000x reference)
"""Trainium2 Bass kernel for nn_ContrastiveDistortion (symmetric pairwise-KL InfoNCE loss).

Math: with IS_SYMMETRIC=True the logdet terms cancel. Let p = 1/sigma^2,
q = mu^2 + sigma^2, m2 = -2*mu*p, pq = p*q. Then (up to per-row constants that
cancel in log-softmax and a uniform +D shift)
  U'[a,b] = p_a.q_b + q_a.p_b + m2_a.mu_b + mu_a.m2_b + colsum(pq)[b]
and logits = -SCL*U' with SCL = 1/(4*T). Five K=128 matmul chunks per output
tile (vs 7 in the naive trace/quad split). Each of the 8 cores gets the full
[128,4096] feature-major mu/sigma in bf16, column-ROTATED by 512*k so the
program is SPMD-identical: the core's own 512-row block is local columns
0..511 (diagonal masked there via an extra (1e30*I, onehot) matmul chunk) and
the positive pairs are local columns 2048..2559.

Walrus allows one sync wait per matmul, so each matmul's (lhsT, rhs) pair must
be written by a single engine. Split by half: h0 planes (needed immediately)
are Act(squares)+DVE(recip/tt); h1 rhs slabs s2/s3 are built by the otherwise
idle Pool engine, with Pool-copied [128,512] lhsT mirrors for those matmuls.
Row-min reductions read PSUM directly on DVE; Act computes
exp(-SCL*(U'-min)) with a row-sum accumulator. The tiny per-row logaddexp tail
runs on host in float64 from the [128,20] (mrow8|esum8|upos4) output.
"""

import sys
from contextlib import ExitStack

import numpy as np

sys.path.insert(0, "/opt/trn_rl_repo")

import concourse.bass as bass
import concourse.bacc as bacc_mod
import concourse.mybir as mybir
from concourse.bass_utils import run_bass_kernel_spmd
from concourse.tile import TileContext

F32 = mybir.dt.float32
BF16 = mybir.dt.bfloat16
I32 = mybir.dt.int32
AF = mybir.ActivationFunctionType
ALU = mybir.AluOpType
AX = mybir.AxisListType

P = 128          # partitions / feature dim D
NB = 4096        # N = 2B rows
NC = 8           # cores
RB = NB // NC    # 512 rows per core
NM = RB // P     # 4 m-chunks of 128 rows
HALF = NB // 2   # 2048 columns per softmax half
TEMPERATURE = 0.1
WEIGHT = 5.0
SCL = 1.0 / (4.0 * TEMPERATURE)  # 2.5: l = -SCL*U' + const_row
BIG = 1e30

# DMA/prep slab splits within each 2048-col half (small first slabs so the
# first matmul group can start early). jj blocks 0..3 map to slabs 0,1,2,2.
SLABS = [(0, 512), (512, 1024), (1024, 2048)]
JJ_SLAB = [0, 1, 2, 2]


def _build_nc():
    nc = bacc_mod.Bacc(None, target_bir_lowering=False, name="contrastive_distortion")
    muT_d = nc.declare_dram_parameter("muT", [P, NB], BF16, isOutput=False)
    sgT_d = nc.declare_dram_parameter("sigmaT", [P, NB], BF16, isOutput=False)
    # out columns: 0:8 = row-min of U' per (h,m); 8:16 = esum per (h,m);
    # 16:20 = positive U' per m
    out_d = nc.declare_dram_parameter("out", [P, 20], F32, isOutput=True)

    with TileContext(nc) as tc, ExitStack() as ctx:
        big = ctx.enter_context(tc.tile_pool(name="big", bufs=1))
        sm = ctx.enter_context(tc.tile_pool(name="sm", bufs=1))
        scr = ctx.enter_context(tc.tile_pool(name="scr", bufs=2))
        pp = ctx.enter_context(tc.tile_pool(name="pp", bufs=2, space="PSUM"))

        # persistent planes, feature-major [128, 4096] bf16
        mu = big.tile([P, NB], BF16)    # DMA-written
        sg = big.tile([P, NB], BF16)    # DMA-written
        var = big.tile([P, NB], BF16)   # Act: sg^2
        msq = big.tile([P, NB], BF16)   # Act: mu^2
        p_ = big.tile([P, NB], BF16)    # DVE: 1/var
        q_ = big.tile([P, NB], BF16)    # DVE h0/h1s1; Pool h1s2/s3: msq+var
        mun2 = big.tile([P, NB], BF16)  # DVE: -2*mu
        m2 = big.tile([P, NB], BF16)    # mun2 * p = -2*mu*p
        muv = big.tile([P, NB], BF16)   # copy of mu
        pq = big.tile([P, NB], BF16)    # p * q
        pc = big.tile([P, NB - HALF], BF16)  # Pool copy of p_ for h1 s2/s3 rhs
        oneh = big.tile([P, RB * NM], F32)    # [128,2048] m-stripe one-hots
        onehb = big.tile([P, RB * NM], BF16)  # bf16 one-hots (mask rhs)

        # Pool-written [128,512] lhsT mirrors (for Pool-written rhs slabs)
        pL = sm.tile([P, RB], BF16)
        qL = sm.tile([P, RB], BF16)
        m2L = sm.tile([P, RB], BF16)
        muvL = sm.tile([P, RB], BF16)

        ioti = sm.tile([P, RB], I32)
        iotP = sm.tile([P, P], I32)
        ones_d = sm.tile([P, P], BF16)  # DVE memset (seed lhsT, DVE rhs)
        ones_p = sm.tile([P, P], BF16)  # Pool memset (seed lhsT, Pool rhs)
        bigIf = sm.tile([P, P], BF16)   # DVE identity
        bigI = sm.tile([P, P], BF16)    # DVE: identity * 1e30
        bias8 = sm.tile([P, 8], F32)
        pm4 = sm.tile([P, NM], F32)
        out20 = sm.tile([P, 20], F32)

        # ---- setup (overlaps input DMA latency) ----
        nc.gpsimd.iota(ioti, pattern=[[1, RB]], base=0, channel_multiplier=-1)
        nc.gpsimd.iota(iotP, pattern=[[1, P]], base=0, channel_multiplier=-1)
        nc.gpsimd.memset(ones_p, 1.0)
        nc.vector.memset(ones_d, 1.0)
        with nc.allow_low_precision("bf16 planes feed the PE"):
            for m in range(NM):
                # oneh_m[p, c] = (c - p == 128*m)
                nc.vector.tensor_single_scalar(
                    out=oneh[:, RB * m:RB * (m + 1)], in_=ioti, scalar=P * m,
                    op=ALU.is_equal)
                nc.vector.tensor_single_scalar(
                    out=onehb[:, RB * m:RB * (m + 1)], in_=ioti, scalar=P * m,
                    op=ALU.is_equal)
            nc.vector.tensor_single_scalar(out=bigIf, in_=iotP, scalar=0,
                                           op=ALU.is_equal)
            nc.vector.tensor_scalar_mul(bigI, bigIf, BIG)

        # ---- input DMAs: sg before mu (recip chain is longest) ----
        for h in range(2):
            for (a, b) in SLABS:
                sl = slice(HALF * h + a, HALF * h + b)
                nc.sync.dma_start(out=sg[:, sl], in_=sgT_d[:, sl])
                nc.sync.dma_start(out=mu[:, sl], in_=muT_d[:, sl])

        # ---- plane prep ----
        with nc.allow_low_precision("bf16 planes feed the PE"):
            # Act: squares for both halves, slab-wise
            for h in range(2):
                for (a, b) in SLABS:
                    sl = slice(HALF * h + a, HALF * h + b)
                    nc.scalar.activation(out=var[:, sl], in_=sg[:, sl],
                                         func=AF.Square)
                    nc.scalar.activation(out=msq[:, sl], in_=mu[:, sl],
                                         func=AF.Square)

            def dve_slab(sl, full):
                nc.vector.reciprocal(p_[:, sl], var[:, sl])
                nc.vector.tensor_scalar_mul(mun2[:, sl], mu[:, sl], -2.0)
                if full:
                    nc.vector.tensor_add(q_[:, sl], msq[:, sl], var[:, sl])
                    nc.vector.tensor_mul(pq[:, sl], p_[:, sl], q_[:, sl])
                    nc.vector.tensor_mul(m2[:, sl], mun2[:, sl], p_[:, sl])
                    nc.vector.tensor_copy(out=muv[:, sl], in_=mu[:, sl])

            # DVE: all of h0, plus h1 s1 and the h1 p/mun2 feeds
            for (a, b) in SLABS:
                dve_slab(slice(a, b), True)
            dve_slab(slice(HALF, HALF + 512), True)
            dve_slab(slice(HALF + 512, HALF + 1024), False)
            dve_slab(slice(HALF + 1024, HALF + 2048), False)

            # Pool: lhsT mirrors (written once h0 s1 planes exist)
            nc.gpsimd.tensor_copy(out=pL, in_=p_[:, 0:RB])
            nc.gpsimd.tensor_copy(out=qL, in_=q_[:, 0:RB])
            nc.gpsimd.tensor_copy(out=m2L, in_=m2[:, 0:RB])
            nc.gpsimd.tensor_copy(out=muvL, in_=muv[:, 0:RB])
            # Pool: h1 s2/s3 rhs planes
            for (a, b) in SLABS[1:]:
                sl = slice(HALF + a, HALF + b)
                cl = slice(a, b)  # pc is [P, 2048] indexed by h1-local col
                nc.gpsimd.tensor_add(q_[:, sl], msq[:, sl], var[:, sl])
                nc.gpsimd.tensor_copy(out=pc[:, cl], in_=p_[:, sl])
                nc.gpsimd.tensor_copy(out=muv[:, sl], in_=mu[:, sl])
                nc.gpsimd.tensor_mul(m2[:, sl], mun2[:, sl], p_[:, sl])
                nc.gpsimd.tensor_mul(pq[:, sl], p_[:, sl], q_[:, sl])

        c8 = 0
        s512_list = []
        for h in range(2):
            for m in range(NM):
                mblk = slice(P * m, P * (m + 1))
                u = pp.tile([P, HALF], F32, name=f"u{h}{m}", tag="ps")
                # A psum-slot-reusing group head needs WAR deps vs the old
                # tile's readers, but matmuls can carry only ONE sync wait in
                # walrus codegen. Orphan bf16 ldweights (no PSUM write -> no
                # PE wait) absorb those deps: the exp's accum_out write (Act)
                # and, for h=1 tiles, the pos-extract scratch write (DVE).
                if c8 >= 2:
                    ec = 2 * (8 + c8 - 2)
                    nc.tensor.ldweights(out20.bitcast(BF16)[0:1, ec:ec + 2])
                if c8 >= 6:
                    nc.tensor.ldweights(
                        s512_list[c8 - 6].bitcast(BF16)[0:1, 0:2])
                for jj in range(4):
                    osl = slice(RB * jj, RB * (jj + 1))
                    gsl = slice(HALF * h + RB * jj, HALF * h + RB * (jj + 1))
                    has_mask = (h == 0 and jj == 0)
                    pool_side = (h == 1 and jj >= 1)
                    if pool_side:
                        chunks = [(pL[:, mblk], q_[:, gsl]),
                                  (qL[:, mblk], pc[:, osl]),
                                  (m2L[:, mblk], muv[:, gsl]),
                                  (muvL[:, mblk], m2[:, gsl]),
                                  (ones_p, pq[:, gsl])]
                    else:
                        chunks = [(p_[:, mblk], q_[:, gsl]),
                                  (q_[:, mblk], pc[:, osl] if h else p_[:, gsl]),
                                  (m2[:, mblk], muv[:, gsl]),
                                  (muv[:, mblk], m2[:, gsl]),
                                  (ones_d, pq[:, gsl])]
                        if h == 1:
                            # h1 jj0: rhs slab s1 is DVE-written, p too
                            chunks[1] = (q_[:, mblk], p_[:, gsl])
                    for ci, (lhsT, rhs) in enumerate(chunks):
                        nc.tensor.matmul(
                            u[:, osl], lhsT=lhsT, rhs=rhs,
                            start=(ci == 0),
                            stop=(ci == len(chunks) - 1 and not has_mask))
                    if has_mask:
                        # diagonal (always in local cols 128m..128m+127):
                        # += 1e30 at (p, 128m+p) so it loses the min and
                        # underflows the exp.
                        nc.tensor.matmul(
                            u[:, osl], lhsT=bigI,
                            rhs=onehb[:, RB * m:RB * (m + 1)],
                            start=False, stop=True)

                # ---- consumers (read PSUM directly) ----
                if h == 1:
                    # positive logits live at local cols 128m+p of this tile
                    s512 = scr.tile([P, RB], F32, name="s512", tag="s512",
                                    bufs=2)
                    s512_list.append(s512)
                    nc.vector.tensor_mul(s512, u[:, 0:RB],
                                         oneh[:, RB * m:RB * (m + 1)])
                    nc.vector.tensor_reduce(out20[:, 16 + m:17 + m], s512,
                                            axis=AX.X, op=ALU.add)
                if c8 == 7:
                    # last tile: per-jj partial mins shorten the drain
                    for jj in range(4):
                        osl = slice(RB * jj, RB * (jj + 1))
                        nc.vector.tensor_reduce(pm4[:, jj:jj + 1], u[:, osl],
                                                axis=AX.X, op=ALU.min)
                    nc.vector.tensor_reduce(out20[:, c8:c8 + 1], pm4,
                                            axis=AX.X, op=ALU.min)
                else:
                    nc.vector.tensor_reduce(out20[:, c8:c8 + 1], u,
                                            axis=AX.X, op=ALU.min)
                nc.vector.tensor_scalar_mul(bias8[:, c8:c8 + 1],
                                            out20[:, c8:c8 + 1], SCL)
                e2k = scr.tile([P, HALF], BF16, name="e2k", tag="e2k", bufs=2)
                nc.scalar.activation(
                    out=e2k, in_=u, func=AF.Exp, bias=bias8[:, c8:c8 + 1],
                    scale=-SCL, accum_out=out20[:, 8 + c8:9 + c8])
                c8 += 1

        nc.sync.dma_start(out=out_d[:, :], in_=out20)

    return nc


_NC_CACHE = None


def _get_nc():
    global _NC_CACHE
    if _NC_CACHE is None:
        nc = _build_nc()
        nc.finalize()  # runs Bacc.compile(): wait legalization for TRN2
        _NC_CACHE = nc
    return _NC_CACHE


def run_sharded(mu_x, sigma_x, mu_p, sigma_p, trace=False):
    import ml_dtypes
    bf16 = ml_dtypes.bfloat16
    mus = np.concatenate([np.asarray(mu_x, np.float32),
                          np.asarray(mu_p, np.float32)], 0)
    sigmas = np.concatenate([np.asarray(sigma_x, np.float32),
                             np.asarray(sigma_p, np.float32)], 0)
    muT = np.ascontiguousarray(mus.T.astype(bf16))
    sgT = np.ascontiguousarray(sigmas.T.astype(bf16))
    in_maps = [
        {"muT": np.ascontiguousarray(np.roll(muT, -RB * k, axis=1)),
         "sigmaT": np.ascontiguousarray(np.roll(sgT, -RB * k, axis=1))}
        for k in range(NC)
    ]
    kwargs = {}
    if trace:
        kwargs = dict(trace=True, trace_cores=[0])
    br = run_bass_kernel_spmd(_get_nc(), in_maps, core_ids=list(range(NC)),
                              **kwargs)
    # host tail in float64: per-row logaddexp over the two halves
    total = 0.0
    for r in br.results:
        o = r["out"].astype(np.float64)       # [128, 20]
        mrow = o[:, 0:8]                      # [p, h*4+m] row-min of U'
        esum = o[:, 8:16]                     # sum of exp(-SCL*(U'-mrow))
        upos = o[:, 16:20]                    # positive U' per (p, m)
        L = -SCL * mrow + np.log(esum)        # [p, 8]
        L0, L1 = L[:, 0:4], L[:, 4:8]
        lse = np.logaddexp(L0, L1)            # [p, m]
        total += float(np.sum(lse + SCL * upos))
    n_classes = NB - 1
    to_mult = (n_classes - 1.0 / WEIGHT) / (n_classes - 1)
    to_add = -np.log(np.float64(to_mult))
    loss = np.float32(total / NB - to_add)
    return loss, br


def kernel(z_hat, mu_x, sigma_x, mu_p, sigma_p):
    loss, _ = run_sharded(mu_x, sigma_x, mu_p, sigma_p)
    return np.asarray(loss, np.float32)
